# revision 6
# baseline (speedup 1.0000x reference)
"""BiAttention (BiDAF) Trainium2 Bass kernel — 8 NeuronCores, sequence-
parallel over the context axis.

kernel(context [16384,100] f32, question [4096,100] f32, kernel [300] f32)
  -> G [16384, 400] f32  (concat: ctx | U_A | ctx*U_A | ctx*H_A)

Single fused pass per core (2048 ctx rows): the softmax stability offset
m_i = c1_i + max(q2) cancels c1 in the S matmul, so S' = cw3.q + (q2 -
max q2) needs only 101 contraction rows and no on-device row-max
pre-pass.  Per 512-ctx tile, 32 S matmuls (fp32r, q-major) feed ACT exp
straight from PSUM; exp tiles drive both the UA accumulation matmul and
a DVE running-max.  The exact row-max is recovered as mhat + ln(maxexp),
folded as ee = maxexp * exp(c1 - OFF) without any ln.  Q2C needs no
collective: each core emits a 101-float partial sum (exp-weighted ctx +
denominator); the host adds the 8 partials, divides, and assembles the
ctx and ctx*H_A output blocks (the former is the input verbatim).
"""
import sys

sys.path.insert(0, "/opt/trn_rl_repo")
from contextlib import ExitStack

import numpy as np

import concourse.bass as bass
import concourse.tile as tile
from concourse import mybir


def split_multi_waits(nc):
    """This walrus build rejects instructions with >1 sync wait. Hoist extra
    waits onto single-wait EventSemaphore nops on the same engine (engines
    execute in order, so N sequential single waits == one N-way wait)."""
    n_split = 0
    counter = [0]

    def make_nop(engine, wait):
        counter[0] += 1
        inst = mybir.InstEventSemaphore(
            name=f"I-waitsplit-{counter[0]}", ins=[], outs=[])
        inst.engine = engine
        inst.sync_info = mybir.SyncInfo(on_wait=[wait], on_update=[])
        return inst

    for f in nc.m.functions:
        for blk in f.blocks:
            changed = False
            new_insts = []
            for inst in blk.instructions:
                si = inst.sync_info
                if si is not None and si.on_wait and len(si.on_wait) > 1:
                    waits = list(si.on_wait)
                    for w in waits[:-1]:
                        new_insts.append(make_nop(inst.engine, w))
                    si.on_wait = [waits[-1]]
                    n_split += 1
                    changed = True
                new_insts.append(inst)
            if changed:
                blk.instructions[:] = new_insts
    return n_split


F32 = mybir.dt.float32
F32R = mybir.dt.float32r
EXP = mybir.ActivationFunctionType.Exp
MULT = mybir.AluOpType.mult

N_CORES = 8
D = 100
R = 2048          # ctx rows per core
M = 4096          # question rows
P = 128           # partitions
NCH = R // P      # 16 ctx chunks
QC = M // P       # 32 q chunks
NT = R // 512     # 4 ctx tiles
CPT = 512 // P    # 4 chunks per ctx tile
C_OFF = 10.0      # Q2C softmax offset headroom above est. global row-max

# load slicing: qaugT/qn in chunk-groups, front-loaded small first
Q_SLICES = [(0, 4), (4, 12), (12, 20), (20, 32)]   # q-chunk ranges


def build_bass():
    nc = bass.Bass("TRN2", target_bir_lowering=False, debug=False,
                   num_devices=N_CORES)
    ctxT_d = nc.dram_tensor("ctxT", [101, R], F32R, kind="ExternalInput").ap()
    qT_d = nc.dram_tensor("qT", [101, M], F32R, kind="ExternalInput").ap()
    qn_d = nc.dram_tensor("qn", [P, QC * 101], F32R,
                          kind="ExternalInput").ap()
    ctxn_d = nc.dram_tensor("ctxn", [P, NCH * 102], F32,
                            kind="ExternalInput").ap()
    id_d = nc.dram_tensor("ident", [P, P], F32, kind="ExternalInput").ap()
    g2_out = nc.dram_tensor("g2", [R, 2 * D], F32, kind="ExternalOutput").ap()
    hl_out = nc.dram_tensor("hl", [101, 1], F32, kind="ExternalOutput").ap()

    with tile.TileContext(nc) as tc:
        with ExitStack() as ex:
            build_body(nc, tc, ex, ctxT_d, qT_d, qn_d, ctxn_d, id_d,
                       g2_out, hl_out)
    return nc


def build_body(nc, tc, ex, ctxT_d, qT_d, qn_d, ctxn_d, id_d, g2_out, hl_out):
    sing = ex.enter_context(tc.tile_pool(name="sing", bufs=1))
    ptt_pool = ex.enter_context(tc.tile_pool(name="ptt", bufs=3))
    macc_pool = ex.enter_context(tc.tile_pool(name="macc", bufs=2))
    uat_pool = ex.enter_context(tc.tile_pool(name="uat", bufs=2))
    uan_pool = ex.enter_context(tc.tile_pool(name="uan", bufs=2))
    g12_pool = ex.enter_context(tc.tile_pool(name="g12", bufs=3))
    # PSUM: stp 3 + uap 2 + tpp 2 + hlp 1 = 8 banks
    stp = ex.enter_context(tc.tile_pool(name="stp", bufs=3, space="PSUM"))
    uap = ex.enter_context(tc.tile_pool(name="uap", bufs=2, space="PSUM"))
    tpp = ex.enter_context(tc.tile_pool(name="tpp", bufs=2, space="PSUM"))
    hlp = ex.enter_context(tc.tile_pool(name="hlp", bufs=1, space="PSUM"))

    # ---- persistent SBUF ----
    caugT = [sing.tile([101, 512], F32R, name=f"caugT{t}") for t in range(NT)]
    qaugT = [sing.tile([101, 128 * (b - a)], F32R, name=f"qaugT{i}")
             for i, (a, b) in enumerate(Q_SLICES)]
    qn = [sing.tile([P, b - a, 101], F32R, name=f"qn{i}")
          for i, (a, b) in enumerate(Q_SLICES)]
    ctxn = [sing.tile([P, 8, 102], F32, name=f"ctxn{h}") for h in range(2)]
    tid = sing.tile([P, P], F32)
    f = sing.tile([P, NCH], F32)
    rmx = sing.tile([P, NCH], F32)
    ee = sing.tile([P, NCH], F32)
    rzs = sing.tile([P, NCH], F32)
    hls = sing.tile([101, 1], F32)
    dummy = sing.tile([1, 1], F32)

    def q_slice(qc):
        for i, (a, b) in enumerate(Q_SLICES):
            if qc < b:
                return i, qc - a
        raise AssertionError

    # ---- input loads, prioritized for earliest PE start ----
    nc.sync.dma_start(out=qaugT[0][:], in_=qT_d[:, 0:512])
    nc.sync.dma_start(out=caugT[0][:], in_=ctxT_d[:, 0:512])
    nc.sync.dma_start(out=qn[0][:], in_=qn_d[:, 0:4 * 101])
    nc.sync.dma_start(out=tid[:], in_=id_d[:])
    nc.sync.dma_start(out=qaugT[1][:], in_=qT_d[:, 512:1536])
    nc.sync.dma_start(out=qn[1][:], in_=qn_d[:, 4 * 101:12 * 101])
    nc.sync.dma_start(out=ctxn[0][:], in_=ctxn_d[:, 0:8 * 102])
    nc.sync.dma_start(out=caugT[1][:], in_=ctxT_d[:, 512:1024])
    nc.sync.dma_start(out=qaugT[2][:], in_=qT_d[:, 1536:2560])
    nc.sync.dma_start(out=qaugT[3][:], in_=qT_d[:, 2560:4096])
    nc.sync.dma_start(out=qn[2][:], in_=qn_d[:, 12 * 101:20 * 101])
    nc.sync.dma_start(out=qn[3][:], in_=qn_d[:, 20 * 101:32 * 101])
    nc.sync.dma_start(out=caugT[2][:], in_=ctxT_d[:, 1024:1536])
    nc.sync.dma_start(out=caugT[3][:], in_=ctxT_d[:, 1536:2048])
    nc.sync.dma_start(out=ctxn[1][:], in_=ctxn_d[:, 8 * 102:16 * 102])

    # preload the exp table set early; f = exp(c1 - OFF) per ctx row
    nc.vector.memset(dummy[:], 0.0)
    nc.scalar.activation(dummy[:], dummy[:], EXP)
    nc.scalar.activation(f[:, 0:8], ctxn[0][:, :, 101], EXP)
    nc.scalar.activation(f[:, 8:16], ctxn[1][:, :, 101], EXP)

    hlps = hlp.tile([101, 1], F32, tag="hlps")

    # per-chunk streams, software-pipelined with one-chunk PE lookahead:
    # PE order: S(0), S(1), UA(0), S(2), UA(1), ...
    state = {}

    def emit_s(t, qc):
        stps = stp.tile([P, 512], F32, tag="stps", name=f"st_{t}_{qc}")
        si, off = q_slice(qc)
        nc.tensor.matmul(stps[:], qaugT[si][:, off * 128:(off + 1) * 128],
                         caugT[t][:], start=True, stop=True)
        ptt = ptt_pool.tile([P, 512], F32R, tag="ptt", name=f"ptt_{t}_{qc}")
        nc.scalar.activation(ptt[:], stps[:], EXP)
        state[(t, qc)] = ptt

    def emit_ua_max(t, qc):
        ptt = state.pop((t, qc))
        nc.tensor.matmul(state[("uaps", t)][:], qn_ap(qc), ptt[:],
                         start=(qc == 0), stop=(qc == QC - 1))
        macc = state[("macc", t)]
        if qc == 0:
            nc.vector.tensor_copy(macc[:], ptt[:])
        else:
            nc.vector.tensor_max(macc[:], macc[:], ptt[:])

    def qn_ap(qc):
        si, off = q_slice(qc)
        return qn[si][:, off, :]

    def start_tile(t):
        state[("uaps", t)] = uap.tile([101, 512], F32, tag="uaps",
                                      name=f"uaps_{t}")
        state[("macc", t)] = macc_pool.tile([P, 512], F32, tag="macc",
                                            name=f"macc_{t}")

    def finish_tile(t):
        uaps = state.pop(("uaps", t))
        macc = state.pop(("macc", t))
        uat = uat_pool.tile([101, 512], F32, tag="uat", name=f"uat_{t}")
        nc.vector.tensor_copy(uat[:], uaps[:])
        uan = uan_pool.tile([P, CPT, 101], F32, tag="uan", name=f"uan_{t}")
        for ci in range(CPT):
            cc = t * CPT + ci
            ch, cco = ctxn[cc // 8], cc % 8
            # exact row-max weight: ee = maxexp * exp(c1 - OFF)
            tpm = tpp.tile([P, P], F32, tag="tp", name=f"tpm_{t}_{ci}")
            nc.tensor.transpose(tpm[:], macc[:, ci * P:(ci + 1) * P], tid[:])
            nc.vector.reduce_max(rmx[:, cc:cc + 1], tpm[:],
                                 axis=mybir.AxisListType.X)
            nc.vector.tensor_mul(ee[:, cc:cc + 1], rmx[:, cc:cc + 1],
                                 f[:, cc:cc + 1])
            nc.tensor.matmul(hlps[:], ch[:, cco, 0:101], ee[:, cc:cc + 1],
                             start=(cc == 0), stop=(cc == NCH - 1))
            # U_A natural layout + normalization + G blocks 1,2
            tpu = tpp.tile([P, 101], F32, tag="tp", name=f"tpu_{t}_{ci}")
            nc.tensor.transpose(tpu[:], uat[:, ci * P:(ci + 1) * P],
                                tid[0:101, 0:101])
            nc.vector.tensor_copy(uan[:, ci, :], tpu[:])
            nc.vector.reciprocal(rzs[:, cc:cc + 1], uan[:, ci, 100:101])
            g12 = g12_pool.tile([P, 2 * D], F32, tag="g12",
                                name=f"g12_{t}_{ci}")
            nc.vector.tensor_scalar_mul(g12[:, 0:D], uan[:, ci, 0:D],
                                        rzs[:, cc:cc + 1])
            nc.vector.scalar_tensor_tensor(
                g12[:, D:2 * D], uan[:, ci, 0:D], rzs[:, cc:cc + 1],
                ch[:, cco, 0:D], MULT, MULT)
            nc.sync.dma_start(out=g2_out[cc * P:(cc + 1) * P, :], in_=g12[:])

    # flat chunk sequence with lookahead-1 and deferred tile finish
    seq = [(t, qc) for t in range(NT) for qc in range(QC)]
    start_tile(0)
    emit_s(0, 0)
    for i in range(1, len(seq) + 1):
        if i < len(seq):
            t, qc = seq[i]
            if qc == 0:
                start_tile(t)
            emit_s(t, qc)
            if qc == 4 and t > 0:
                finish_tile(t - 1)
        emit_ua_max(*seq[i - 1])
    finish_tile(NT - 1)

    # Q2C partial out: [sum ee*ctx | sum ee]
    nc.vector.tensor_copy(hls[:], hlps[:])
    nc.sync.dma_start(out=hl_out[:], in_=hls[:])


_nc_cache = None


def _get_nc():
    global _nc_cache
    if _nc_cache is None:
        _nc_cache = build_bass()
        split_multi_waits(_nc_cache)
    return _nc_cache


def _prep_in_maps(inputs):
    context = np.ascontiguousarray(inputs["context"], dtype=np.float32)
    question = np.ascontiguousarray(inputs["question"], dtype=np.float32)
    kern = np.ascontiguousarray(inputs["kernel"], dtype=np.float32)
    w1, w2, w3 = kern[:D], kern[D:2 * D], kern[2 * D:]
    q2 = question @ w2
    maxq2 = float(q2.max())
    c1 = context @ w1
    c1n_all = c1 - float(c1.max()) - C_OFF

    qT = np.empty((101, M), np.float32)
    qT[0:D] = (question * w3[None, :]).T
    qT[D] = q2 - maxq2
    qT = np.ascontiguousarray(qT)
    qn = np.ones((P, QC, 101), np.float32)
    qn[:, :, 0:D] = question.reshape(QC, P, D).transpose(1, 0, 2)
    qn = np.ascontiguousarray(qn.reshape(P, QC * 101))
    ident = np.eye(P, dtype=np.float32)

    in_maps = []
    for k in range(N_CORES):
        cshard = context[k * R:(k + 1) * R]
        ctxT = np.empty((101, R), np.float32)
        ctxT[0:D] = cshard.T
        ctxT[D] = 1.0
        ctxn = np.ones((P, NCH, 102), np.float32)
        ctxn[:, :, 0:D] = cshard.reshape(NCH, P, D).transpose(1, 0, 2)
        ctxn[:, :, 101] = c1n_all[k * R:(k + 1) * R].reshape(NCH, P).T
        in_maps.append({
            "ctxT": np.ascontiguousarray(ctxT),
            "qT": qT,
            "qn": qn,
            "ctxn": np.ascontiguousarray(ctxn.reshape(P, NCH * 102)),
            "ident": ident,
        })
    return context, in_maps


def _assemble(context, results):
    G = np.empty((N_CORES * R, 4 * D), np.float32)
    G[:, 0:D] = context
    hl = np.zeros(101, np.float64)
    for k in range(N_CORES):
        G[k * R:(k + 1) * R, D:3 * D] = results[k]["g2"]
        hl += results[k]["hl"][:, 0].astype(np.float64)
    h = (hl[0:D] / hl[D]).astype(np.float32)
    np.multiply(context, h[None, :], out=G[:, 3 * D:4 * D])
    return G


def kernel(**inputs):
    from concourse.bass_utils import run_bass_kernel_spmd

    context, in_maps = _prep_in_maps(inputs)
    res = run_bass_kernel_spmd(_get_nc(), in_maps,
                               core_ids=list(range(N_CORES)))
    return _assemble(context, res.results)


def kernel_traced(**inputs):
    """Like kernel() but also returns HW exec time in ns (NTFF profile)."""
    from concourse.bass_utils import run_bass_kernel_spmd

    kernel(**inputs)  # warm compile via cached nc
    context, in_maps = _prep_in_maps(inputs)
    res = run_bass_kernel_spmd(_get_nc(), in_maps,
                               core_ids=list(range(N_CORES)), trace=True)
    return _assemble(context, res.results), res.exec_time_ns


# revision 7
# speedup vs baseline: 1.1033x; 1.1033x over previous
"""BiAttention (BiDAF) Trainium2 Bass kernel — 8 NeuronCores, sequence-
parallel over the context axis.

kernel(context [16384,100] f32, question [4096,100] f32, kernel [300] f32)
  -> G [16384, 400] f32  (concat: ctx | U_A | ctx*U_A | ctx*H_A)

Single fused pass per core (2048 ctx rows): the softmax stability offset
m_i = c1_i + max(q2) cancels c1 in the S matmul, so S' = cw3.q + (q2 -
max q2) needs only 101 contraction rows and no on-device row-max
pre-pass.  Per 512-ctx tile, 32 S matmuls (fp32r, q-major) feed ACT exp
straight from PSUM; exp tiles drive both the UA accumulation matmul and
a DVE running-max.  The exact row-max is recovered as mhat + ln(maxexp),
folded as ee = maxexp * exp(c1 - OFF) without any ln.  Q2C needs no
collective: each core emits a 101-float partial sum (exp-weighted ctx +
denominator); the host adds the 8 partials, divides, and assembles the
ctx and ctx*H_A output blocks (the former is the input verbatim).

Inputs are pre-sliced host-side into contiguous per-slice DRAM tensors,
DMA'd with 2D access patterns only (3D APs explode into sub-512B
descriptors), split across the SP and ACT hardware DGE queues so the
first matmul issues ~1.5us in.  Tile-finish work is batched (one
reduce/copy per 4 chunks) and emission-staggered so the in-order PE
queue never waits on DVE results.
"""
import sys

sys.path.insert(0, "/opt/trn_rl_repo")
from contextlib import ExitStack

import numpy as np

import concourse.bass as bass
import concourse.tile as tile
from concourse import mybir


def split_multi_waits(nc):
    """This walrus build rejects instructions with >1 sync wait. Hoist extra
    waits onto single-wait EventSemaphore nops on the same engine (engines
    execute in order, so N sequential single waits == one N-way wait)."""
    n_split = 0
    counter = [0]

    def make_nop(engine, wait):
        counter[0] += 1
        inst = mybir.InstEventSemaphore(
            name=f"I-waitsplit-{counter[0]}", ins=[], outs=[])
        inst.engine = engine
        inst.sync_info = mybir.SyncInfo(on_wait=[wait], on_update=[])
        return inst

    for f in nc.m.functions:
        for blk in f.blocks:
            changed = False
            new_insts = []
            for inst in blk.instructions:
                si = inst.sync_info
                if si is not None and si.on_wait and len(si.on_wait) > 1:
                    waits = list(si.on_wait)
                    for w in waits[:-1]:
                        new_insts.append(make_nop(inst.engine, w))
                    si.on_wait = [waits[-1]]
                    n_split += 1
                    changed = True
                new_insts.append(inst)
            if changed:
                blk.instructions[:] = new_insts
    return n_split


F32 = mybir.dt.float32
F32R = mybir.dt.float32r
EXP = mybir.ActivationFunctionType.Exp
MULT = mybir.AluOpType.mult

N_CORES = 8
D = 100
R = 2048          # ctx rows per core
M = 4096          # question rows
P = 128           # partitions
NCH = R // P      # 16 ctx chunks
QC = M // P       # 32 q chunks
NT = R // 512     # 4 ctx tiles
CPT = 512 // P    # 4 chunks per ctx tile
C_OFF = 10.0      # Q2C softmax offset headroom above est. global row-max

# q-chunk load slices, front-loaded small first
Q_SLICES = [(0, 4), (4, 12), (12, 20), (20, 32)]


def build_bass():
    nc = bass.Bass("TRN2", target_bir_lowering=False, debug=False,
                   num_devices=N_CORES)
    ctxT_d = [nc.dram_tensor(f"ctxT{t}", [101, 512], F32R,
                             kind="ExternalInput").ap() for t in range(NT)]
    qT_d = [nc.dram_tensor(f"qT{i}", [101, 128 * (b - a)], F32R,
                           kind="ExternalInput").ap()
            for i, (a, b) in enumerate(Q_SLICES)]
    qn_d = [nc.dram_tensor(f"qn{i}", [P, 101 * (b - a)], F32R,
                           kind="ExternalInput").ap()
            for i, (a, b) in enumerate(Q_SLICES)]
    ctxn_d = [nc.dram_tensor(f"ctxn{h}", [P, 8 * 102], F32,
                             kind="ExternalInput").ap() for h in range(2)]
    id_d = nc.dram_tensor("ident", [P, P], F32, kind="ExternalInput").ap()
    g2_out = nc.dram_tensor("g2", [R, 2 * D], F32, kind="ExternalOutput").ap()
    hl_out = nc.dram_tensor("hl", [101, 1], F32, kind="ExternalOutput").ap()

    with tile.TileContext(nc) as tc:
        with ExitStack() as ex:
            build_body(nc, tc, ex, ctxT_d, qT_d, qn_d, ctxn_d, id_d,
                       g2_out, hl_out)
    return nc


def build_body(nc, tc, ex, ctxT_d, qT_d, qn_d, ctxn_d, id_d, g2_out, hl_out):
    sing = ex.enter_context(tc.tile_pool(name="sing", bufs=1))
    ptt_pool = ex.enter_context(tc.tile_pool(name="ptt", bufs=3))
    macc_pool = ex.enter_context(tc.tile_pool(name="macc", bufs=2))
    uat_pool = ex.enter_context(tc.tile_pool(name="uat", bufs=2))
    uan_pool = ex.enter_context(tc.tile_pool(name="uan", bufs=2))
    g12_pool = ex.enter_context(tc.tile_pool(name="g12", bufs=3))
    # PSUM: stp 3 + uap 2 + tpp 2 + hlp 1 = 8 banks
    stp = ex.enter_context(tc.tile_pool(name="stp", bufs=3, space="PSUM"))
    uap = ex.enter_context(tc.tile_pool(name="uap", bufs=2, space="PSUM"))
    tpp = ex.enter_context(tc.tile_pool(name="tpp", bufs=2, space="PSUM"))
    hlp = ex.enter_context(tc.tile_pool(name="hlp", bufs=1, space="PSUM"))

    # ---- persistent SBUF (2D tiles only: 3D DMA APs shatter descriptors)
    caugT = [sing.tile([101, 512], F32R, name=f"caugT{t}") for t in range(NT)]
    qaugT = [sing.tile([101, 128 * (b - a)], F32R, name=f"qaugT{i}")
             for i, (a, b) in enumerate(Q_SLICES)]
    qn = [sing.tile([P, 101 * (b - a)], F32R, name=f"qn{i}")
          for i, (a, b) in enumerate(Q_SLICES)]
    ctxn = [sing.tile([P, 8 * 102], F32, name=f"ctxn{h}") for h in range(2)]
    tid = sing.tile([P, P], F32)
    f = sing.tile([P, NCH], F32)
    rmx = sing.tile([P, NCH], F32)
    ee = sing.tile([P, NCH], F32)
    rzs = sing.tile([P, NCH], F32)
    hls = sing.tile([101, 1], F32)
    dummy = sing.tile([1, 1], F32)

    def q_slice(qc):
        for i, (a, b) in enumerate(Q_SLICES):
            if qc < b:
                return i, qc - a
        raise AssertionError

    def qn_ap(qc):
        si, off = q_slice(qc)
        return qn[si][:, off * 101:(off + 1) * 101]

    def ctx_ap(cc, w):
        return ctxn[cc // 8][:, (cc % 8) * 102:(cc % 8) * 102 + w]

    # ---- input loads: SP queue gets caugT + late qT, ACT queue the rest
    nc.sync.dma_start(out=caugT[0][:], in_=ctxT_d[0][:])
    nc.scalar.dma_start(out=qaugT[0][:], in_=qT_d[0][:])
    nc.sync.dma_start(out=qaugT[1][:], in_=qT_d[1][:])
    nc.scalar.dma_start(out=qn[0][:], in_=qn_d[0][:])
    nc.scalar.dma_start(out=qn[1][:], in_=qn_d[1][:])
    nc.sync.dma_start(out=qaugT[2][:], in_=qT_d[2][:])
    nc.sync.dma_start(out=qaugT[3][:], in_=qT_d[3][:])
    nc.scalar.dma_start(out=qn[2][:], in_=qn_d[2][:])
    nc.scalar.dma_start(out=qn[3][:], in_=qn_d[3][:])
    nc.sync.dma_start(out=caugT[1][:], in_=ctxT_d[1][:])
    nc.scalar.dma_start(out=ctxn[0][:], in_=ctxn_d[0][:])
    nc.sync.dma_start(out=caugT[2][:], in_=ctxT_d[2][:])
    nc.scalar.dma_start(out=ctxn[1][:], in_=ctxn_d[1][:])
    nc.sync.dma_start(out=caugT[3][:], in_=ctxT_d[3][:])
    nc.scalar.dma_start(out=tid[:], in_=id_d[:])

    # preload the exp table set; f = exp(c1 - OFF) per ctx row
    nc.vector.memset(dummy[:], 0.0)
    nc.scalar.activation(dummy[:], dummy[:], EXP)
    for h in range(2):
        c1v = ctxn[h][:].rearrange("p (c k) -> p c k", k=102)[:, :, 101]
        nc.scalar.activation(f[:, h * 8:(h + 1) * 8], c1v, EXP)

    hlps = hlp.tile([101, 1], F32, tag="hlps")
    state = {}

    def emit_s(t, qc):
        stps = stp.tile([P, 512], F32, tag="stps", name=f"st_{t}_{qc}")
        si, off = q_slice(qc)
        nc.tensor.matmul(stps[:], qaugT[si][:, off * 128:(off + 1) * 128],
                         caugT[t][:], start=True, stop=True)
        ptt = ptt_pool.tile([P, 512], F32R, tag="ptt", name=f"ptt_{t}_{qc}")
        nc.scalar.activation(ptt[:], stps[:], EXP)
        state[(t, qc)] = ptt

    def emit_ua_max(t, qc):
        ptt = state.pop((t, qc))
        nc.tensor.matmul(state[("uaps", t)][:], qn_ap(qc), ptt[:],
                         start=(qc == 0), stop=(qc == QC - 1))
        macc = state[("macc", t)]
        if qc == 0:
            nc.vector.tensor_copy(macc[:], ptt[:])
        else:
            nc.vector.tensor_max(macc[:], macc[:], ptt[:])

    def start_tile(t):
        state[("uaps", t)] = uap.tile([101, 512], F32, tag="uaps",
                                      name=f"uaps_{t}")
        state[("macc", t)] = macc_pool.tile([P, 512], F32, tag="macc",
                                            name=f"macc_{t}")

    # tile-finish, staggered into the next tile's chunk stream so the
    # in-order PE queue never waits on DVE results:
    #   A (qc==2): maxexp transposes + rowmax reduce + ee
    #   B (qc==6): hl matmuls (ee now ready) + uat copy
    #   C (qc==10): U_A transposes, normalization, G blocks 1+2, DMA out
    def finish_a(t):
        macc = state[("macc", t)]
        tpm = tpp.tile([P, 4, P], F32, tag="tp", name=f"tpm_{t}")
        for ci in range(CPT):
            nc.tensor.transpose(tpm[:, ci, :], macc[:, ci * P:(ci + 1) * P],
                                tid[:])
        sl = slice(t * CPT, (t + 1) * CPT)
        nc.vector.reduce_max(rmx[:, sl], tpm[:], axis=mybir.AxisListType.X)
        nc.vector.tensor_mul(ee[:, sl], rmx[:, sl], f[:, sl])

    def finish_b(t):
        state.pop(("macc", t))
        for ci in range(CPT):
            cc = t * CPT + ci
            nc.tensor.matmul(hlps[:], ctx_ap(cc, 101), ee[:, cc:cc + 1],
                             start=(cc == 0), stop=(cc == NCH - 1))
        uaps = state.pop(("uaps", t))
        uat = uat_pool.tile([101, 512], F32, tag="uat", name=f"uat_{t}")
        nc.vector.tensor_copy(uat[:], uaps[:])
        state[("uat", t)] = uat

    def finish_c(t):
        uat = state.pop(("uat", t))
        tpu = tpp.tile([P, 4, 101], F32, tag="tp", name=f"tpu_{t}")
        for ci in range(CPT):
            nc.tensor.transpose(tpu[:, ci, :], uat[:, ci * P:(ci + 1) * P],
                                tid[0:101, 0:101])
        uan = uan_pool.tile([P, 4, 101], F32, tag="uan", name=f"uan_{t}")
        nc.vector.tensor_copy(uan[:], tpu[:])
        sl = slice(t * CPT, (t + 1) * CPT)
        nc.vector.reciprocal(rzs[:, sl], uan[:, :, 100])
        for ci in range(CPT):
            cc = t * CPT + ci
            g12 = g12_pool.tile([P, 2 * D], F32, tag="g12",
                                name=f"g12_{t}_{ci}")
            nc.vector.tensor_scalar_mul(g12[:, 0:D], uan[:, ci, 0:D],
                                        rzs[:, cc:cc + 1])
            nc.vector.scalar_tensor_tensor(
                g12[:, D:2 * D], uan[:, ci, 0:D], rzs[:, cc:cc + 1],
                ctx_ap(cc, D), MULT, MULT)
            nc.sync.dma_start(out=g2_out[cc * P:(cc + 1) * P, :], in_=g12[:])

    # flat chunk sequence with lookahead-1 PE emission
    seq = [(t, qc) for t in range(NT) for qc in range(QC)]
    start_tile(0)
    emit_s(0, 0)
    for i in range(1, len(seq) + 1):
        if i < len(seq):
            t, qc = seq[i]
            if qc == 0:
                start_tile(t)
            emit_s(t, qc)
            if t > 0:
                if qc == 2:
                    finish_a(t - 1)
                elif qc == 6:
                    finish_b(t - 1)
                elif qc == 10:
                    finish_c(t - 1)
        emit_ua_max(*seq[i - 1])
    t = NT - 1
    finish_a(t)
    finish_b(t)
    finish_c(t)

    # Q2C partial out: [sum ee*ctx | sum ee]
    nc.vector.tensor_copy(hls[:], hlps[:])
    nc.sync.dma_start(out=hl_out[:], in_=hls[:])


_nc_cache = None


def _get_nc():
    global _nc_cache
    if _nc_cache is None:
        _nc_cache = build_bass()
        split_multi_waits(_nc_cache)
    return _nc_cache


def _prep_in_maps(inputs):
    context = np.ascontiguousarray(inputs["context"], dtype=np.float32)
    question = np.ascontiguousarray(inputs["question"], dtype=np.float32)
    kern = np.ascontiguousarray(inputs["kernel"], dtype=np.float32)
    w1, w2, w3 = kern[:D], kern[D:2 * D], kern[2 * D:]
    q2 = question @ w2
    maxq2 = float(q2.max())
    c1 = context @ w1
    c1n_all = c1 - float(c1.max()) - C_OFF

    qT = np.empty((101, M), np.float32)
    qT[0:D] = (question * w3[None, :]).T
    qT[D] = q2 - maxq2
    qnf = np.ones((P, QC, 101), np.float32)
    qnf[:, :, 0:D] = question.reshape(QC, P, D).transpose(1, 0, 2)
    shared = {}
    for i, (a, b) in enumerate(Q_SLICES):
        shared[f"qT{i}"] = np.ascontiguousarray(qT[:, a * P:b * P])
        shared[f"qn{i}"] = np.ascontiguousarray(
            qnf[:, a:b, :].reshape(P, (b - a) * 101))
    shared["ident"] = np.eye(P, dtype=np.float32)

    in_maps = []
    for k in range(N_CORES):
        cshard = context[k * R:(k + 1) * R]
        ctxT = np.empty((101, R), np.float32)
        ctxT[0:D] = cshard.T
        ctxT[D] = 1.0
        ctxnf = np.ones((P, NCH, 102), np.float32)
        ctxnf[:, :, 0:D] = cshard.reshape(NCH, P, D).transpose(1, 0, 2)
        ctxnf[:, :, 101] = c1n_all[k * R:(k + 1) * R].reshape(NCH, P).T
        m = dict(shared)
        for t in range(NT):
            m[f"ctxT{t}"] = np.ascontiguousarray(ctxT[:, t * 512:(t + 1) * 512])
        for h in range(2):
            m[f"ctxn{h}"] = np.ascontiguousarray(
                ctxnf[:, h * 8:(h + 1) * 8, :].reshape(P, 8 * 102))
        in_maps.append(m)
    return context, in_maps


def _assemble(context, results):
    G = np.empty((N_CORES * R, 4 * D), np.float32)
    G[:, 0:D] = context
    hl = np.zeros(101, np.float64)
    for k in range(N_CORES):
        G[k * R:(k + 1) * R, D:3 * D] = results[k]["g2"]
        hl += results[k]["hl"][:, 0].astype(np.float64)
    h = (hl[0:D] / hl[D]).astype(np.float32)
    np.multiply(context, h[None, :], out=G[:, 3 * D:4 * D])
    return G


def kernel(**inputs):
    from concourse.bass_utils import run_bass_kernel_spmd

    context, in_maps = _prep_in_maps(inputs)
    res = run_bass_kernel_spmd(_get_nc(), in_maps,
                               core_ids=list(range(N_CORES)))
    return _assemble(context, res.results)


def kernel_traced(**inputs):
    """Like kernel() but also returns HW exec time in ns (NTFF profile)."""
    from concourse.bass_utils import run_bass_kernel_spmd

    kernel(**inputs)  # warm compile via cached nc
    context, in_maps = _prep_in_maps(inputs)
    res = run_bass_kernel_spmd(_get_nc(), in_maps,
                               core_ids=list(range(N_CORES)), trace=True)
    return _assemble(context, res.results), res.exec_time_ns
